# revision 36
# baseline (speedup 1.0000x reference)
"""Multi-head attention (dense transformer) kernel for Trainium2, 8 NeuronCores.

Sharding (batch x head-block): core c handles batch b = c//2 and the 8 heads
[8*(c%2), 8*(c%2)+8) of 16.  Per the tensor-parallel hint, the projection
weights are sliced per-core host-side (column slices of Wq/Wk/Wv, row slices
of Wo, pre-transposed into the layout the PE array consumes); the Wo partial
products of the two cores sharing a batch are summed host-side on unshard
(row-parallel all-reduce equivalent).

Device algorithm per core (all matmuls in float32r on the PE array):
  - transpose x_q/x_k/x_v (seq, d) -> (d, seq) via PE-transpose tiles
  - QT = (Wq/8 slice) @ xq.T   -> (i, t) per head, with one spare partition row
    per head for the softmax-normalizer row ("augmented" row)
  - KT likewise, augmented with a constant ones row
  - V  = xv @ Wv.T slice       -> (s, i) natural
  - S  = QT_h.T @ KT_h (t, s) blocks (causal only) + mask on diag blocks,
    exp on ACT with per-row accumulation -> row sums, reciprocal, normalized
    weights DMA'd out
  - ST = KTa_h.T @ QTa_h (s, t) blocks with K=65: the augmented row adds
    -ln(rowsum[t]) to every score so exp() directly yields *normalized*
    attention weights transposed -- no per-free-dim rescale needed
  - OT = sum_s V_h.T-packed @ exp(ST)  -> (i, t), two heads column-packed
  - out_partial = OT.T @ WoT -> (t, o) natural, DMA'd out
"""

import sys

try:
    import concourse  # noqa: F401  (already on PYTHONPATH in the axon env)
except ImportError:
    sys.path.insert(0, "/opt/trn_rl_repo")

import numpy as np

import concourse.bacc as bacc
import concourse.bass as bass
import concourse.mybir as mybir
from concourse.masks import make_identity
from concourse.tile import TileContext

P = 128
T = 1024          # tgt == src sequence length
D = 1024          # d_model
DK = 64           # head dim
HPC = 8           # heads per core
IC = HPC * DK     # 512 projected dims per core
NT = T // P       # 8 seq tiles
ND = D // P       # 8 d_model chunks
F32 = mybir.dt.float32
F32R = mybir.dt.float32r
AX = mybir.AxisListType.X
EXP = mybir.ActivationFunctionType.Exp
LN = mybir.ActivationFunctionType.Ln


def _r(ap):
    return ap.bitcast(F32R)


def _t_groups(start):
    """512-wide groups covering [start, T)."""
    out = []
    while start < T:
        w = min(512, T - start)
        out.append((start, w))
        start += w
    return out


def _t_groups_aligned(start):
    """Groups covering [start, T) that never cross a 512 (psum bank)
    boundary in absolute coordinates."""
    out = []
    while start < T:
        end = min(T, (start // 512 + 1) * 512)
        out.append((start, end - start))
        start = end
    return out


def _interleave(*gens):
    """Round-robin emission from several unit-generators.  Only safe when the
    generators are data-independent: Tile dependencies follow emission order,
    so a consumer unit emitted before its producer reads stale memory."""
    live = [g for g in gens]
    while live:
        for g in list(live):
            try:
                next(g)
            except StopIteration:
                live.remove(g)


def _interleave_lagged(producer, consumer, lag):
    """Round-robin after emitting `lag` producer units first, keeping every
    consumer unit behind the producer units it reads from."""
    for _ in range(lag):
        try:
            next(producer)
        except StopIteration:
            break
    _interleave(producer, consumer)


def _build_body(nc, tc, dram):
    xq, xk, xv, wqt, wkt, wvt, wot, cmask, cmaskt, w_out, o_part = dram

    import contextlib

    with contextlib.ExitStack() as ctx:
        consts = ctx.enter_context(tc.tile_pool(name="consts", bufs=1))
        persist = ctx.enter_context(tc.tile_pool(name="persist", bufs=1))

        ident = consts.tile([P, P], F32)
        make_identity(nc, ident)
        cm = consts.tile([P, P], F32)
        nc.sync.dma_start(out=cm, in_=cmask[:, :])
        cmt = consts.tile([P, P], F32)
        nc.sync.dma_start(out=cmt, in_=cmaskt[:, :])

        # persistent per-core tensors
        QA = persist.tile([P, HPC, T], F32)   # head h: rows 0:64 = q~ (i, t), row 64 = -ln(rowsum)
        KA = persist.tile([P, HPC, T], F32)   # head h: rows 0:64 = k (i, s), row 64 = ones
        V = persist.tile([P, NT, IC], F32)    # (s_local, s_tile, i)
        OT = persist.tile([P, 4, T], F32)     # (i_local, i_chunk, t)
        # 1/rowsum per (t_local, head*NT + t_tile), flat for clean [P, 1] slices
        RECIP = persist.tile([P, HPC * NT], F32)

        # PSUM budget is 8 banks, statically split and phase-swapped:
        #   setup span:  ps_mm x2 (transposes/projections) + ps_sst x2 + av x4
        #   tail span:   ps_sst x2 + av x4 + ps_wo x2 (output projection)
        with tc.tile_pool(name="upool", bufs=4) as upool, \
             tc.tile_pool(name="utpool", bufs=3) as utpool, \
             tc.tile_pool(name="stats", bufs=6) as stats, \
             tc.tile_pool(name="ps_sst", bufs=2, space="PSUM") as ps_sst:

            ps_av = None  # opened after the setup pools release their banks

            setup_ctx = contextlib.ExitStack()
            xnat = setup_ctx.enter_context(tc.tile_pool(name="xnat", bufs=2))
            xtp = setup_ctx.enter_context(tc.tile_pool(name="xtp", bufs=1))
            wstage = setup_ctx.enter_context(tc.tile_pool(name="wstage", bufs=1))
            ps_mm = setup_ctx.enter_context(
                tc.tile_pool(name="ps_mm", bufs=4, space="PSUM"))

            def load_transpose(xdram, name):
                """load X natural tiles, PE-transpose into (d, t) layout.
                Returns (XT_tile, emission_generator)."""
                XT = xtp.tile([P, ND, T], F32, tag="xt", name=f"XT_{name}")

                def gen():
                    for i in range(NT):
                        xn = xnat.tile([P, D], F32, tag="xn", name=f"xn_{name}{i}")
                        nc.sync.dma_start(out=xn, in_=xdram[i * P:(i + 1) * P, :])
                        for j in range(ND):
                            pt = ps_mm.tile([P, P], F32, tag="mm",
                                              name=f"pt{name}{i}{j}")
                            nc.tensor.transpose(pt, xn[:, j * P:(j + 1) * P], ident)
                            nc.any.tensor_copy(_r(XT[:, j, i * P:(i + 1) * P]), pt)
                        yield

                return XT, gen()

            def load_w(wdram, name):
                W = wstage.tile([P, ND, IC], F32, tag="w", name=f"W_{name}")
                nc.sync.dma_start(
                    out=_r(W), in_=_r(wdram.rearrange("(c p) i -> p c i", p=P))
                )
                return W

            def proj_qk(W, XT, dst):
                # per head: its 64 out dims land at psum partitions 0:64
                for hh in range(HPC):
                    for th in range(2):
                        ps = ps_mm.tile([P, 512], F32, tag="mm", name=f"pj{hh}{th}")
                        for j in range(ND):
                            nc.tensor.matmul(
                                ps[0:DK, :],
                                _r(W[:, j, hh * DK:(hh + 1) * DK]),
                                _r(XT[:, j, th * 512:(th + 1) * 512]),
                                start=(j == 0), stop=(j == ND - 1),
                            )
                        nc.any.tensor_copy(
                            _r(dst[0:DK, hh, th * 512:(th + 1) * 512]), ps[0:DK, :]
                        )
                    yield

            def proj_v(W, XT):
                for si in range(NT):
                    ps = ps_mm.tile([P, 512], F32, tag="mm", name=f"pv{si}")
                    for j in range(ND):
                        nc.tensor.matmul(
                            ps,
                            _r(XT[:, j, si * P:(si + 1) * P]),
                            _r(W[:, j, :]),
                            start=(j == 0), stop=(j == ND - 1),
                        )
                    nc.any.tensor_copy(_r(V[:, si, :]), ps)
                    yield

            def s_phase(pi):
                """scores (t, s) + softmax stats + normalized weight output
                for heads 2*pi, 2*pi+1."""
                for hp in range(2):
                    hh = 2 * pi + hp
                    for ti in range(NT):
                        wid = P * (ti + 1)
                        u = upool.tile([P, T], F32, tag="u")
                        pss = ps_sst.tile([P, T], F32, tag="sst",
                                          name=f"s{hh}_{ti}")
                        for (s0, w0) in _t_groups(0):
                            if s0 >= wid:
                                break
                            w = min(w0, wid - s0)
                            nc.tensor.matmul(
                                pss[:, s0:s0 + w],
                                _r(QA[0:DK, hh, ti * P:(ti + 1) * P]),
                                _r(KA[0:DK, hh, s0:s0 + w]),
                                start=True, stop=True,
                            )
                        # diagonal block: additive causal mask
                        nc.vector.tensor_tensor(
                            pss[:, wid - P:wid], pss[:, wid - P:wid], cm,
                            op=mybir.AluOpType.add,
                        )
                        rs = stats.tile([P, 1], F32, tag="rs")
                        nc.scalar.activation(
                            u[:, 0:wid], pss[:, 0:wid], EXP, accum_out=rs
                        )
                        rc = RECIP[:, hh * NT + ti:hh * NT + ti + 1]
                        nc.vector.reciprocal(rc, rs)
                        nc.gpsimd.tensor_scalar_mul(u[:, 0:wid], u[:, 0:wid], rc)
                        # split weight stores between the SWDGE (Pool) and
                        # HWDGE (SP) rings to halve per-ring DMA issue cost
                        eng = nc.gpsimd if hp == 0 else nc.sync
                        eng.dma_start(
                            out=w_out[hh, ti * P:(ti + 1) * P, 0:wid],
                            in_=u[:, 0:wid],
                        )
                        yield

            def b_phase(pi):
                """augmented rows, transposed scores, AV accumulation for pair pi."""
                # -ln(rowsum) rows: Ln(recip) -> PE transpose -> 1-row DMA per head
                lnr = stats.tile([P, 2 * NT], F32, tag="lnr")
                nc.scalar.activation(lnr, RECIP[:, 2 * pi * NT:(2 * pi + 2) * NT], LN)
                pst = ps_sst.tile([2 * NT, P], F32, tag="sst", name=f"lnt{pi}")
                nc.tensor.transpose(pst, lnr, ident)
                lnt = stats.tile([2 * NT, P], F32, tag="lnts")
                nc.vector.tensor_copy(_r(lnt), pst)
                for hp in range(2):
                    hh = 2 * pi + hp
                    nc.sync.dma_start(
                        out=_r(QA[DK:DK + 1, hh, :]),
                        in_=_r(lnt[hp * NT:(hp + 1) * NT, :]),
                    )
                yield

                # AV accumulator banks.  Column tile_position is illegal for
                # f32r matmuls, so each (t-group, head) pair gets its own
                # accumulation chain at psum partitions 0:64.
                avps = [
                    [
                        ps_av.tile([DK, 512], F32, tag=f"av{g}{hp}",
                                   name=f"av{pi}_{g}_{hp}")
                        for hp in range(2)
                    ]
                    for g in range(2)
                ]

                def emit_ot(g):
                    span = slice(512 * g, 512 * (g + 1))
                    # head even: lane-aligned copy to OT partitions 0:64
                    nc.vector.tensor_copy(_r(OT[0:DK, pi, span]), avps[g][0])
                    # head odd: DVE copy to SBUF staging, then partition-shift
                    # via SBUF->SBUF DMA into OT partitions 64:128
                    otmp = stats.tile([DK, 512], F32, tag="otmp", bufs=2,
                                      name=f"otmp{pi}_{g}")
                    nc.vector.tensor_copy(_r(otmp), avps[g][1])
                    nc.sync.dma_start(out=_r(OT[DK:P, pi, span]), in_=_r(otmp))

                for si in range(NT):
                    uts = []
                    for hp in range(2):
                        hh = 2 * pi + hp
                        span = T - si * P
                        ut = utpool.tile([P, T], F32, tag=f"ut{hp}")
                        ps = ps_sst.tile([P, T], F32, tag="sst",
                                         name=f"st{hh}_{si}")
                        for (t0, w) in _t_groups_aligned(si * P):
                            nc.tensor.matmul(
                                ps[:, t0:t0 + w],
                                _r(KA[0:DK + 1, hh, si * P:(si + 1) * P]),
                                _r(QA[0:DK + 1, hh, t0:t0 + w]),
                                start=True, stop=True,
                            )
                        # diagonal block (transposed mask)
                        nc.vector.tensor_tensor(
                            ps[:, si * P:(si + 1) * P],
                            ps[:, si * P:(si + 1) * P], cmt,
                            op=mybir.AluOpType.add,
                        )
                        nc.scalar.activation(
                            _r(ut[:, 0:span]), ps[:, si * P:T], EXP,
                        )
                        uts.append(ut)
                    # AV: accumulate over s tiles
                    for g in range(si // 4, 2):
                        t0 = max(512 * g, si * P)
                        t1 = 512 * (g + 1)
                        last_si = 3 if g == 0 else 7
                        for hp in range(2):
                            nc.tensor.matmul(
                                avps[g][hp][:, t0 - 512 * g:t1 - 512 * g],
                                _r(V[:, si, pi * P + DK * hp:pi * P + DK * (hp + 1)]),
                                _r(uts[hp][:, t0 - si * P:t1 - si * P]),
                                start=(si == 0),
                                stop=(si == last_si),
                            )
                        if si == last_si:
                            emit_ot(g)  # release the g-chain banks early
                    yield

            # ---- emission order ----
            from itertools import chain as _chain

            XTq, gq = load_transpose(xq, "q")
            for _ in gq:
                pass
            Wq = load_w(wqt, "q")
            for _ in proj_qk(Wq, XTq, QA):
                pass
            # K setup feeds the first score phase head-by-head: lag 10 keeps
            # each s0 unit behind the K projection units it reads
            XTk, gk = load_transpose(xk, "k")
            Wk = load_w(wkt, "k")
            _interleave_lagged(_chain(gk, proj_qk(Wk, XTk, KA)), s_phase(0),
                               lag=10)
            # ones row for the K-side augmented contraction (before any ST)
            nc.vector.memset(KA[DK:DK + 1, :, :], 1.0)

            # V setup overlaps s1 (no data flow between them); b0 then comes
            # strictly after V is fully emitted since its AV units read V
            XTv, gv = load_transpose(xv, "v")
            Wv = load_w(wvt, "v")
            _interleave(_chain(gv, proj_v(Wv, XTv)), s_phase(1))
            setup_ctx.close()
            # AV accumulation banks, opened now that ps_mm's are free
            av_ctx = contextlib.ExitStack()
            ps_av = av_ctx.enter_context(
                tc.tile_pool(name="ps_av", bufs=1, space="PSUM"))

            _interleave(b_phase(0), s_phase(2))
            _interleave(b_phase(1), s_phase(3))
            for _ in b_phase(2):
                pass
            for _ in b_phase(3):
                pass
            av_ctx.close()

        # ---------------- phase C: output projection ----------------
        with tc.tile_pool(name="osb", bufs=3) as osb, \
             tc.tile_pool(name="wotp", bufs=1) as wotp, \
             tc.tile_pool(name="ps_wo", bufs=2, space="PSUM") as ps_wo:
            WOT = wotp.tile([P, 4, D], F32)   # (i_local, i_chunk, o)
            nc.sync.dma_start(
                out=_r(WOT), in_=_r(wot.rearrange("(c p) o -> p c o", p=P))
            )
            for ti in range(NT):
                o = osb.tile([P, D], F32, tag="o")
                for oh in range(2):
                    ps = ps_wo.tile([P, 512], F32, tag="wo")
                    for pi in range(4):
                        nc.tensor.matmul(
                            ps,
                            _r(OT[:, pi, ti * P:(ti + 1) * P]),
                            _r(WOT[:, pi, oh * 512:(oh + 1) * 512]),
                            start=(pi == 0), stop=(pi == 3),
                        )
                    nc.vector.tensor_copy(o[:, oh * 512:(oh + 1) * 512], ps)
                nc.sync.dma_start(
                    out=o_part[ti * P:(ti + 1) * P, :], in_=o,
                )


_NC_CACHE = None


def _get_nc():
    global _NC_CACHE
    if _NC_CACHE is not None:
        return _NC_CACHE
    nc = bacc.Bacc(None, target_bir_lowering=False)
    xq = nc.dram_tensor("xq", [T, D], F32, kind="ExternalInput")
    xk = nc.dram_tensor("xk", [T, D], F32, kind="ExternalInput")
    xv = nc.dram_tensor("xv", [T, D], F32, kind="ExternalInput")
    wqt = nc.dram_tensor("wqt", [D, IC], F32, kind="ExternalInput")
    wkt = nc.dram_tensor("wkt", [D, IC], F32, kind="ExternalInput")
    wvt = nc.dram_tensor("wvt", [D, IC], F32, kind="ExternalInput")
    wot = nc.dram_tensor("wot", [IC, D], F32, kind="ExternalInput")
    cmask = nc.dram_tensor("cmask", [P, P], F32, kind="ExternalInput")
    cmaskt = nc.dram_tensor("cmaskt", [P, P], F32, kind="ExternalInput")
    w_out = nc.dram_tensor("w_out", [HPC, T, T], F32, kind="ExternalOutput")
    o_part = nc.dram_tensor("o_part", [T, D], F32, kind="ExternalOutput")
    dram = (xq[:], xk[:], xv[:], wqt[:], wkt[:], wvt[:], wot[:],
            cmask[:], cmaskt[:], w_out[:], o_part[:])
    with TileContext(nc) as tc:
        _build_body(nc, tc, dram)
    nc.compile()
    _NC_CACHE = nc
    return nc


def make_in_maps(query, key, value, mask, Wq, Wk, Wv, Wo):
    query = np.asarray(query, np.float32)
    key = np.asarray(key, np.float32)
    value = np.asarray(value, np.float32)
    mask = np.asarray(mask, np.float32)
    Wq = np.asarray(Wq, np.float32)
    Wk = np.asarray(Wk, np.float32)
    Wv = np.asarray(Wv, np.float32)
    Wo = np.asarray(Wo, np.float32)
    cmask = np.ascontiguousarray(mask[0:P, 0:P])
    cmaskt = np.ascontiguousarray(cmask.T)
    in_maps = []
    for c in range(8):
        b = c // 2
        r0 = IC * (c % 2)
        in_maps.append({
            "xq": np.ascontiguousarray(query[:, b, :]),
            "xk": np.ascontiguousarray(key[:, b, :]),
            "xv": np.ascontiguousarray(value[:, b, :]),
            "wqt": np.ascontiguousarray((Wq[r0:r0 + IC, :] * 0.125).T),
            "wkt": np.ascontiguousarray(Wk[r0:r0 + IC, :].T),
            "wvt": np.ascontiguousarray(Wv[r0:r0 + IC, :].T),
            "wot": np.ascontiguousarray(Wo[:, r0:r0 + IC].T),
            "cmask": cmask,
            "cmaskt": cmaskt,
        })
    return in_maps


def kernel(query, key, value, mask, padding_mask, Wq, Wk, Wv, Wo, **kwargs):
    from concourse.bass_utils import run_bass_kernel_spmd

    nc = _get_nc()
    in_maps = make_in_maps(query, key, value, mask, Wq, Wk, Wv, Wo)
    res = run_bass_kernel_spmd(nc, in_maps, core_ids=list(range(8)))
    results = res.results
    attn_weights = np.concatenate(
        [results[c]["w_out"] for c in range(8)], axis=0
    )
    attn_output = np.empty((T, 4, D), np.float32)
    for b in range(4):
        attn_output[:, b, :] = results[2 * b]["o_part"] + results[2 * b + 1]["o_part"]
    return attn_output, attn_weights


# revision 39
# speedup vs baseline: 1.0262x; 1.0262x over previous
"""Multi-head attention (dense transformer) kernel for Trainium2, 8 NeuronCores.

Sharding (batch x head-block): core c handles batch b = c//2 and the 8 heads
[8*(c%2), 8*(c%2)+8) of 16.  Per the tensor-parallel hint, the projection
weights are sliced per-core host-side (column slices of Wq/Wk/Wv, row slices
of Wo, pre-transposed into the layout the PE array consumes); the Wo partial
products of the two cores sharing a batch are summed host-side on unshard
(row-parallel all-reduce equivalent).

Device algorithm per core (all matmuls in float32r on the PE array):
  - transpose x_q/x_k/x_v (seq, d) -> (d, seq) via PE-transpose tiles
  - QT = (Wq/8 slice) @ xq.T   -> (i, t) per head, with one spare partition row
    per head for the softmax-normalizer row ("augmented" row)
  - KT likewise, augmented with a constant ones row
  - V  = xv @ Wv.T slice       -> (s, i) natural
  - S  = QT_h.T @ KT_h (t, s) blocks (causal only) + mask on diag blocks,
    exp on ACT with per-row accumulation -> row sums, reciprocal, normalized
    weights DMA'd out
  - ST = KTa_h.T @ QTa_h (s, t) blocks with K=65: the augmented row adds
    -ln(rowsum[t]) to every score so exp() directly yields *normalized*
    attention weights transposed -- no per-free-dim rescale needed
  - OT = sum_s V_h.T-packed @ exp(ST)  -> (i, t), two heads column-packed
  - out_partial = OT.T @ WoT -> (t, o) natural, DMA'd out
"""

import sys

try:
    import concourse  # noqa: F401  (already on PYTHONPATH in the axon env)
except ImportError:
    sys.path.insert(0, "/opt/trn_rl_repo")

import numpy as np

import concourse.bacc as bacc
import concourse.bass as bass
import concourse.mybir as mybir
from concourse.masks import make_identity
from concourse.tile import TileContext

P = 128
T = 1024          # tgt == src sequence length
D = 1024          # d_model
DK = 64           # head dim
HPC = 8           # heads per core
IC = HPC * DK     # 512 projected dims per core
NT = T // P       # 8 seq tiles
ND = D // P       # 8 d_model chunks
F32 = mybir.dt.float32
F32R = mybir.dt.float32r
AX = mybir.AxisListType.X
EXP = mybir.ActivationFunctionType.Exp
LN = mybir.ActivationFunctionType.Ln


def _r(ap):
    return ap.bitcast(F32R)


def _t_groups(start):
    """512-wide groups covering [start, T)."""
    out = []
    while start < T:
        w = min(512, T - start)
        out.append((start, w))
        start += w
    return out


def _t_groups_aligned(start):
    """Groups covering [start, T) that never cross a 512 (psum bank)
    boundary in absolute coordinates."""
    out = []
    while start < T:
        end = min(T, (start // 512 + 1) * 512)
        out.append((start, end - start))
        start = end
    return out


def _interleave(*gens):
    """Round-robin emission from several unit-generators.  Only safe when the
    generators are data-independent: Tile dependencies follow emission order,
    so a consumer unit emitted before its producer reads stale memory."""
    live = [g for g in gens]
    while live:
        for g in list(live):
            try:
                next(g)
            except StopIteration:
                live.remove(g)


def _interleave_lagged(producer, consumer, lag):
    """Round-robin after emitting `lag` producer units first, keeping every
    consumer unit behind the producer units it reads from."""
    for _ in range(lag):
        try:
            next(producer)
        except StopIteration:
            break
    _interleave(producer, consumer)


def _build_body(nc, tc, dram):
    xq, xk, xv, wqt, wkt, wvt, wot, cmask, cmaskt, w_out, o_part = dram

    import contextlib

    with contextlib.ExitStack() as ctx:
        consts = ctx.enter_context(tc.tile_pool(name="consts", bufs=1))
        persist = ctx.enter_context(tc.tile_pool(name="persist", bufs=1))

        ident = consts.tile([P, P], F32)
        make_identity(nc, ident)
        cm = consts.tile([P, P], F32)
        nc.sync.dma_start(out=cm, in_=cmask[:, :])
        cmt = consts.tile([P, P], F32)
        nc.sync.dma_start(out=cmt, in_=cmaskt[:, :])

        # persistent per-core tensors
        QA = persist.tile([P, HPC, T], F32)   # head h: rows 0:64 = q~ (i, t), row 64 = -ln(rowsum)
        KA = persist.tile([P, HPC, T], F32)   # head h: rows 0:64 = k (i, s), row 64 = ones
        V = persist.tile([P, NT, IC], F32)    # (s_local, s_tile, i)
        OT = persist.tile([P, 4, T], F32)     # (i_local, i_chunk, t)
        # 1/rowsum per (t_local, head*NT + t_tile), flat for clean [P, 1] slices
        RECIP = persist.tile([P, HPC * NT], F32)

        # PSUM budget is 8 banks, statically split and phase-swapped:
        #   setup span:  ps_mm x2 (transposes/projections) + ps_sst x2 + av x4
        #   tail span:   ps_sst x2 + av x4 + ps_wo x2 (output projection)
        with tc.tile_pool(name="upool", bufs=4) as upool, \
             tc.tile_pool(name="utpool", bufs=3) as utpool, \
             tc.tile_pool(name="stats", bufs=6) as stats, \
             tc.tile_pool(name="ps_sst", bufs=2, space="PSUM") as ps_sst:

            ps_av = None  # opened after the setup pools release their banks

            setup_ctx = contextlib.ExitStack()
            xnat = setup_ctx.enter_context(tc.tile_pool(name="xnat", bufs=2))
            xtp = setup_ctx.enter_context(tc.tile_pool(name="xtp", bufs=1))
            wstage = setup_ctx.enter_context(tc.tile_pool(name="wstage", bufs=1))
            ps_mm = setup_ctx.enter_context(
                tc.tile_pool(name="ps_mm", bufs=4, space="PSUM"))

            def load_transpose(xdram, name):
                """load X natural tiles, PE-transpose into (d, t) layout.
                Returns (XT_tile, emission_generator)."""
                XT = xtp.tile([P, ND, T], F32, tag="xt", name=f"XT_{name}")

                def gen():
                    for i in range(NT):
                        xn = xnat.tile([P, D], F32, tag="xn", name=f"xn_{name}{i}")
                        nc.sync.dma_start(out=xn, in_=xdram[i * P:(i + 1) * P, :])
                        for j in range(ND):
                            pt = ps_mm.tile([P, P], F32, tag="mm",
                                              name=f"pt{name}{i}{j}")
                            nc.tensor.transpose(pt, xn[:, j * P:(j + 1) * P], ident)
                            nc.any.tensor_copy(_r(XT[:, j, i * P:(i + 1) * P]), pt)
                        yield

                return XT, gen()

            def load_w(wdram, name):
                W = wstage.tile([P, ND, IC], F32, tag="w", name=f"W_{name}")
                nc.sync.dma_start(
                    out=_r(W), in_=_r(wdram.rearrange("(c p) i -> p c i", p=P))
                )
                return W

            def proj_qk(W, XT, dst):
                # per head: its 64 out dims land at psum partitions 0:64
                for hh in range(HPC):
                    for th in range(2):
                        ps = ps_mm.tile([P, 512], F32, tag="mm", name=f"pj{hh}{th}")
                        for j in range(ND):
                            nc.tensor.matmul(
                                ps[0:DK, :],
                                _r(W[:, j, hh * DK:(hh + 1) * DK]),
                                _r(XT[:, j, th * 512:(th + 1) * 512]),
                                start=(j == 0), stop=(j == ND - 1),
                            )
                        nc.any.tensor_copy(
                            _r(dst[0:DK, hh, th * 512:(th + 1) * 512]), ps[0:DK, :]
                        )
                    yield

            def proj_v(W, XT):
                for si in range(NT):
                    ps = ps_mm.tile([P, 512], F32, tag="mm", name=f"pv{si}")
                    for j in range(ND):
                        nc.tensor.matmul(
                            ps,
                            _r(XT[:, j, si * P:(si + 1) * P]),
                            _r(W[:, j, :]),
                            start=(j == 0), stop=(j == ND - 1),
                        )
                    nc.any.tensor_copy(_r(V[:, si, :]), ps)
                    yield

            def s_phase(pi):
                """scores (t, s) + softmax stats + normalized weight output
                for heads 2*pi, 2*pi+1."""
                for hp in range(2):
                    hh = 2 * pi + hp
                    for ti in range(NT):
                        wid = P * (ti + 1)
                        u = upool.tile([P, T], F32, tag="u")
                        pss = ps_sst.tile([P, T], F32, tag="sst",
                                          name=f"s{hh}_{ti}")
                        for (s0, w0) in _t_groups(0):
                            if s0 >= wid:
                                break
                            w = min(w0, wid - s0)
                            nc.tensor.matmul(
                                pss[:, s0:s0 + w],
                                _r(QA[0:DK, hh, ti * P:(ti + 1) * P]),
                                _r(KA[0:DK, hh, s0:s0 + w]),
                                start=True, stop=True,
                            )
                        # diagonal block: additive causal mask
                        nc.vector.tensor_tensor(
                            pss[:, wid - P:wid], pss[:, wid - P:wid], cm,
                            op=mybir.AluOpType.add,
                        )
                        rs = stats.tile([P, 1], F32, tag="rs")
                        nc.scalar.activation(
                            u[:, 0:wid], pss[:, 0:wid], EXP, accum_out=rs
                        )
                        rc = RECIP[:, hh * NT + ti:hh * NT + ti + 1]
                        nc.vector.reciprocal(rc, rs)
                        nc.gpsimd.tensor_scalar_mul(u[:, 0:wid], u[:, 0:wid], rc)
                        # all bulk weight stores go to the SWDGE (Pool)
                        # ring so the SP HWDGE FIFO stays clear for the
                        # latency-critical aug-row and OT-shift transfers
                        eng = nc.gpsimd
                        eng.dma_start(
                            out=w_out[hh, ti * P:(ti + 1) * P, 0:wid],
                            in_=u[:, 0:wid],
                        )
                        yield

            def b_phase(pi):
                """augmented rows, transposed scores, AV accumulation for pair pi."""
                # -ln(rowsum) rows: Ln(recip) -> PE transpose -> 1-row DMA per head
                lnr = stats.tile([P, 2 * NT], F32, tag="lnr")
                nc.scalar.activation(lnr, RECIP[:, 2 * pi * NT:(2 * pi + 2) * NT], LN)
                pst = ps_sst.tile([2 * NT, P], F32, tag="sst", name=f"lnt{pi}")
                nc.tensor.transpose(pst, lnr, ident)
                lnt = stats.tile([2 * NT, P], F32, tag="lnts")
                nc.vector.tensor_copy(_r(lnt), pst)
                for hp in range(2):
                    hh = 2 * pi + hp
                    nc.sync.dma_start(
                        out=_r(QA[DK:DK + 1, hh, :]),
                        in_=_r(lnt[hp * NT:(hp + 1) * NT, :]),
                    )
                yield

                # AV accumulator banks.  Column tile_position is illegal for
                # f32r matmuls, so each (t-group, head) pair gets its own
                # accumulation chain at psum partitions 0:64.
                avps = [
                    [
                        ps_av.tile([DK, 512], F32, tag=f"av{g}{hp}",
                                   name=f"av{pi}_{g}_{hp}")
                        for hp in range(2)
                    ]
                    for g in range(2)
                ]

                def emit_ot(g):
                    span = slice(512 * g, 512 * (g + 1))
                    # head even: lane-aligned copy to OT partitions 0:64
                    nc.vector.tensor_copy(_r(OT[0:DK, pi, span]), avps[g][0])
                    # head odd: DVE copy to SBUF staging, then partition-shift
                    # via SBUF->SBUF DMA into OT partitions 64:128
                    otmp = stats.tile([DK, 512], F32, tag="otmp", bufs=2,
                                      name=f"otmp{pi}_{g}")
                    nc.vector.tensor_copy(_r(otmp), avps[g][1])
                    nc.sync.dma_start(out=_r(OT[DK:P, pi, span]), in_=_r(otmp))

                for si in range(NT):
                    uts = []
                    for hp in range(2):
                        hh = 2 * pi + hp
                        span = T - si * P
                        ut = utpool.tile([P, T], F32, tag=f"ut{hp}")
                        ps = ps_sst.tile([P, T], F32, tag="sst",
                                         name=f"st{hh}_{si}")
                        for (t0, w) in _t_groups_aligned(si * P):
                            nc.tensor.matmul(
                                ps[:, t0:t0 + w],
                                _r(KA[0:DK + 1, hh, si * P:(si + 1) * P]),
                                _r(QA[0:DK + 1, hh, t0:t0 + w]),
                                start=True, stop=True,
                            )
                        # diagonal block (transposed mask)
                        nc.vector.tensor_tensor(
                            ps[:, si * P:(si + 1) * P],
                            ps[:, si * P:(si + 1) * P], cmt,
                            op=mybir.AluOpType.add,
                        )
                        nc.scalar.activation(
                            _r(ut[:, 0:span]), ps[:, si * P:T], EXP,
                        )
                        uts.append(ut)
                    # AV: accumulate over s tiles
                    for g in range(si // 4, 2):
                        t0 = max(512 * g, si * P)
                        t1 = 512 * (g + 1)
                        last_si = 3 if g == 0 else 7
                        for hp in range(2):
                            nc.tensor.matmul(
                                avps[g][hp][:, t0 - 512 * g:t1 - 512 * g],
                                _r(V[:, si, pi * P + DK * hp:pi * P + DK * (hp + 1)]),
                                _r(uts[hp][:, t0 - si * P:t1 - si * P]),
                                start=(si == 0),
                                stop=(si == last_si),
                            )
                        if si == last_si:
                            emit_ot(g)  # release the g-chain banks early
                    yield

            # ---- emission order ----
            from itertools import chain as _chain

            XTq, gq = load_transpose(xq, "q")
            for _ in gq:
                pass
            Wq = load_w(wqt, "q")
            for _ in proj_qk(Wq, XTq, QA):
                pass
            # K setup feeds the first score phase head-by-head: lag 10 keeps
            # each s0 unit behind the K projection units it reads
            XTk, gk = load_transpose(xk, "k")
            Wk = load_w(wkt, "k")
            _interleave_lagged(_chain(gk, proj_qk(Wk, XTk, KA)), s_phase(0),
                               lag=10)
            # ones row for the K-side augmented contraction (before any ST)
            nc.vector.memset(KA[DK:DK + 1, :, :], 1.0)

            # V setup overlaps s1 (no data flow between them); b0 then comes
            # strictly after V is fully emitted since its AV units read V
            XTv, gv = load_transpose(xv, "v")
            Wv = load_w(wvt, "v")
            _interleave(_chain(gv, proj_v(Wv, XTv)), s_phase(1))
            setup_ctx.close()
            # AV accumulation banks, opened now that ps_mm's are free
            av_ctx = contextlib.ExitStack()
            ps_av = av_ctx.enter_context(
                tc.tile_pool(name="ps_av", bufs=1, space="PSUM"))

            _interleave(b_phase(0), s_phase(2))
            _interleave(b_phase(1), s_phase(3))
            for _ in b_phase(2):
                pass
            for _ in b_phase(3):
                pass
            av_ctx.close()

        # ---------------- phase C: output projection ----------------
        with tc.tile_pool(name="osb", bufs=3) as osb, \
             tc.tile_pool(name="wotp", bufs=1) as wotp, \
             tc.tile_pool(name="ps_wo", bufs=2, space="PSUM") as ps_wo:
            WOT = wotp.tile([P, 4, D], F32)   # (i_local, i_chunk, o)
            nc.sync.dma_start(
                out=_r(WOT), in_=_r(wot.rearrange("(c p) o -> p c o", p=P))
            )
            for ti in range(NT):
                o = osb.tile([P, D], F32, tag="o")
                for oh in range(2):
                    ps = ps_wo.tile([P, 512], F32, tag="wo")
                    for pi in range(4):
                        nc.tensor.matmul(
                            ps,
                            _r(OT[:, pi, ti * P:(ti + 1) * P]),
                            _r(WOT[:, pi, oh * 512:(oh + 1) * 512]),
                            start=(pi == 0), stop=(pi == 3),
                        )
                    nc.vector.tensor_copy(o[:, oh * 512:(oh + 1) * 512], ps)
                nc.sync.dma_start(
                    out=o_part[ti * P:(ti + 1) * P, :], in_=o,
                )


_NC_CACHE = None


def _get_nc():
    global _NC_CACHE
    if _NC_CACHE is not None:
        return _NC_CACHE
    nc = bacc.Bacc(None, target_bir_lowering=False)
    xq = nc.dram_tensor("xq", [T, D], F32, kind="ExternalInput")
    xk = nc.dram_tensor("xk", [T, D], F32, kind="ExternalInput")
    xv = nc.dram_tensor("xv", [T, D], F32, kind="ExternalInput")
    wqt = nc.dram_tensor("wqt", [D, IC], F32, kind="ExternalInput")
    wkt = nc.dram_tensor("wkt", [D, IC], F32, kind="ExternalInput")
    wvt = nc.dram_tensor("wvt", [D, IC], F32, kind="ExternalInput")
    wot = nc.dram_tensor("wot", [IC, D], F32, kind="ExternalInput")
    cmask = nc.dram_tensor("cmask", [P, P], F32, kind="ExternalInput")
    cmaskt = nc.dram_tensor("cmaskt", [P, P], F32, kind="ExternalInput")
    w_out = nc.dram_tensor("w_out", [HPC, T, T], F32, kind="ExternalOutput")
    o_part = nc.dram_tensor("o_part", [T, D], F32, kind="ExternalOutput")
    dram = (xq[:], xk[:], xv[:], wqt[:], wkt[:], wvt[:], wot[:],
            cmask[:], cmaskt[:], w_out[:], o_part[:])
    with TileContext(nc) as tc:
        _build_body(nc, tc, dram)
    nc.compile()
    _NC_CACHE = nc
    return nc


def make_in_maps(query, key, value, mask, Wq, Wk, Wv, Wo):
    query = np.asarray(query, np.float32)
    key = np.asarray(key, np.float32)
    value = np.asarray(value, np.float32)
    mask = np.asarray(mask, np.float32)
    Wq = np.asarray(Wq, np.float32)
    Wk = np.asarray(Wk, np.float32)
    Wv = np.asarray(Wv, np.float32)
    Wo = np.asarray(Wo, np.float32)
    cmask = np.ascontiguousarray(mask[0:P, 0:P])
    cmaskt = np.ascontiguousarray(cmask.T)
    in_maps = []
    for c in range(8):
        b = c // 2
        r0 = IC * (c % 2)
        in_maps.append({
            "xq": np.ascontiguousarray(query[:, b, :]),
            "xk": np.ascontiguousarray(key[:, b, :]),
            "xv": np.ascontiguousarray(value[:, b, :]),
            "wqt": np.ascontiguousarray((Wq[r0:r0 + IC, :] * 0.125).T),
            "wkt": np.ascontiguousarray(Wk[r0:r0 + IC, :].T),
            "wvt": np.ascontiguousarray(Wv[r0:r0 + IC, :].T),
            "wot": np.ascontiguousarray(Wo[:, r0:r0 + IC].T),
            "cmask": cmask,
            "cmaskt": cmaskt,
        })
    return in_maps


def kernel(query, key, value, mask, padding_mask, Wq, Wk, Wv, Wo, **kwargs):
    from concourse.bass_utils import run_bass_kernel_spmd

    nc = _get_nc()
    in_maps = make_in_maps(query, key, value, mask, Wq, Wk, Wv, Wo)
    res = run_bass_kernel_spmd(nc, in_maps, core_ids=list(range(8)))
    results = res.results
    attn_weights = np.concatenate(
        [results[c]["w_out"] for c in range(8)], axis=0
    )
    attn_output = np.empty((T, 4, D), np.float32)
    for b in range(4):
        attn_output[:, b, :] = results[2 * b]["o_part"] + results[2 * b + 1]["o_part"]
    return attn_output, attn_weights
